# revision 1
# baseline (speedup 1.0000x reference)
"""Trainium2 Bass kernel for nn_MPCActor: MLP (256->512->512->32, relu/relu/
sigmoid) followed by 100 SGD steps on u (closed form, since the per-element
recurrence u <- a*u + b with a = 1-2*lr*q, b = -lr*p has the exact solution
u_N = a^N u0 - 0.5*(p/q)*(1 - a^N)).

Data parallel over 8 NeuronCores: batch 32768 -> 4096 rows per core, MLP
weights replicated. Activations are kept feature-on-partition / batch-on-free
so weights serve as the stationary matmul operand in their natural [in, out]
layout; obs tiles are transposed on the PE. Matmuls run in bf16 (fp32
accumulate in PSUM); everything after the sigmoid stays fp32.

Only the 8 W3 columns that the u-update actually reads (q_u = cols 12:16,
p_u = cols 28:32) are computed; x_init never enters the gradient.

Engine split per batch tile: PE transposes + matmuls; PSUM drains alternate
between ACT (relu w/ bias) and DVE (fused add-bias+max0 tensor_scalar);
the f32->bf16 obs cast runs on the otherwise idle GpSimd; layer 2 runs its
K-chunk loop outermost so its matmuls start as soon as the first y1 chunk
is drained.
"""

import numpy as np

import concourse.bass as bass
import concourse.mybir as mybir
import concourse.tile as tile
from concourse import bacc, masks
from concourse.bass_utils import run_bass_kernel_spmd

NCORES = 8
BATCH = 32768
BPC = BATCH // NCORES  # 4096 rows per core
OBS = 256
HID = 512
NQP = 8  # q_u (4) + p_u (4) columns of W3 that matter
BT = 512  # batch tile (matmul moving free dim)
NT = BPC // BT  # 8 batch tiles per core
LR = 0.01
F32 = mybir.dt.float32
MD = mybir.dt.bfloat16  # matmul dtype

_CACHE = {}


def _build_nc():
    nc = bacc.Bacc(
        trn_type="TRN2", target_bir_lowering=False, debug=False, num_devices=NCORES
    )
    obs = nc.declare_dram_parameter("obs", [BPC, OBS], F32, isOutput=False).ap()
    u0 = nc.declare_dram_parameter("u0", [BPC, 4], F32, isOutput=False).ap()
    w1 = nc.declare_dram_parameter("w1", [OBS, HID], F32, isOutput=False).ap()
    w2 = nc.declare_dram_parameter("w2", [HID, HID], F32, isOutput=False).ap()
    w3 = nc.declare_dram_parameter("w3", [HID, NQP], F32, isOutput=False).ap()
    b1 = nc.declare_dram_parameter("b1", [128, 4], F32, isOutput=False).ap()
    b2 = nc.declare_dram_parameter("b2", [128, 4], F32, isOutput=False).ap()
    b3 = nc.declare_dram_parameter("b3", [NQP, 1], F32, isOutput=False).ap()
    uo = nc.declare_dram_parameter("uo", [BPC, 4], F32, isOutput=True).ap()

    AF = mybir.ActivationFunctionType
    ALU = mybir.AluOpType

    with tile.TileContext(nc) as tc:
        from contextlib import ExitStack

        with ExitStack() as ctx:
            singles = ctx.enter_context(tc.tile_pool(name="singles", bufs=1))
            p_obsf = ctx.enter_context(tc.tile_pool(name="obsf", bufs=2))
            p_obsb = ctx.enter_context(tc.tile_pool(name="obsb", bufs=2))
            p_obsT = ctx.enter_context(tc.tile_pool(name="obsT", bufs=2))
            p_y1 = ctx.enter_context(tc.tile_pool(name="y1", bufs=2))
            p_y2 = ctx.enter_context(tc.tile_pool(name="y2", bufs=2))
            p_qp = ctx.enter_context(tc.tile_pool(name="qp", bufs=2))
            p_cf = ctx.enter_context(tc.tile_pool(name="cf", bufs=2))
            # PSUM budget is 8 banks: ot 2 + y1 2 + y2 2 + z3 1 + qpt 1
            pp_ot = ctx.enter_context(tc.tile_pool(name="ppot", bufs=2, space="PSUM"))
            pp_y1 = ctx.enter_context(tc.tile_pool(name="ppy1", bufs=2, space="PSUM"))
            pp_y2 = ctx.enter_context(tc.tile_pool(name="ppy2", bufs=2, space="PSUM"))
            pp_z3 = ctx.enter_context(tc.tile_pool(name="ppz3", bufs=1, space="PSUM"))
            pp_qpt = ctx.enter_context(tc.tile_pool(name="ppqpt", bufs=1, space="PSUM"))

            # ---- one-time: weights (cast to bf16), biases, identities ----
            w1f = singles.tile([128, 2, HID], F32)
            nc.sync.dma_start(out=w1f, in_=w1.rearrange("(kc p) m -> p kc m", p=128))
            w1s = singles.tile([128, 2, HID], MD)
            nc.vector.tensor_copy(out=w1s, in_=w1f)

            w2f = singles.tile([128, 4, HID], F32)
            nc.sync.dma_start(out=w2f, in_=w2.rearrange("(kc p) m -> p kc m", p=128))
            w2s = singles.tile([128, 4, HID], MD)
            nc.vector.tensor_copy(out=w2s, in_=w2f)

            w3f = singles.tile([128, 4, NQP], F32)
            nc.sync.dma_start(out=w3f, in_=w3.rearrange("(kc p) m -> p kc m", p=128))
            w3s = singles.tile([128, 4, NQP], MD)
            nc.vector.tensor_copy(out=w3s, in_=w3f)

            b1s = singles.tile([128, 4], F32)
            nc.sync.dma_start(out=b1s, in_=b1)
            b2s = singles.tile([128, 4], F32)
            nc.sync.dma_start(out=b2s, in_=b2)
            b3s = singles.tile([NQP, 1], F32)
            nc.sync.dma_start(out=b3s, in_=b3)

            ident = singles.tile([128, 128], MD)
            masks.make_identity(nc, ident[:])
            id8 = singles.tile([8, 8], F32)
            masks.make_identity(nc, id8[:])

            obs_t = obs.rearrange("(t c p) f -> t p c f", p=128, c=4)
            u0_t = u0.rearrange("(t c p) j -> p t c j", p=128, c=4)
            uo_t = uo.rearrange("(t c p) j -> p t c j", p=128, c=4)


            def drain(dst, src, bias_ap, m):
                if m % 2 == 0:
                    nc.scalar.activation(
                        out=dst, in_=src, func=AF.Relu, bias=bias_ap, scale=1.0
                    )
                else:
                    nc.vector.tensor_scalar(dst, src, bias_ap, 0.0, ALU.add, ALU.max)

            for it in range(NT):
                # load obs tile [128, 4, 256]; cast on GpSimd
                obsf = p_obsf.tile([128, 4, OBS], F32)
                nc.sync.dma_start(out=obsf, in_=obs_t[it])
                obsb = p_obsb.tile([128, 4, OBS], MD)
                nc.vector.tensor_copy(out=obsb, in_=obsf)

                # transpose to obsT [256, BT] as 2 chunks of [128, BT]
                obsT = []
                for f in range(2):
                    ps = pp_ot.tile([128, BT], MD, tag="ot")
                    for c in range(4):
                        nc.tensor.transpose(
                            ps[:, c * 128 : (c + 1) * 128],
                            obsb[:, c, f * 128 : (f + 1) * 128],
                            ident[:],
                        )
                    ot = p_obsT.tile([128, BT], MD, tag=f"obsT{f}")
                    nc.vector.tensor_copy(out=ot, in_=ps)
                    obsT.append(ot)

                # layer 1: y1T[m] = relu(W1[:, m].T @ obsT + b1[m])
                y1 = []
                for m in range(4):
                    ps = pp_y1.tile([128, BT], F32, tag="y1")
                    for kc in range(2):
                        nc.tensor.matmul(
                            ps,
                            w1s[:, kc, m * 128 : (m + 1) * 128],
                            obsT[kc],
                            start=(kc == 0),
                            stop=(kc == 1),
                        )
                    t = p_y1.tile([128, BT], MD, tag=f"y1_{m}")
                    drain(t, ps, b1s[:, m : m + 1], m)
                    y1.append(t)

                # layer 2
                y2 = []
                for m in range(4):
                    ps = pp_y2.tile([128, BT], F32, name="ps2", tag="y2")
                    for kc in range(4):
                        nc.tensor.matmul(
                            ps,
                            w2s[:, kc, m * 128 : (m + 1) * 128],
                            y1[kc],
                            start=(kc == 0),
                            stop=(kc == 3),
                        )
                    t = p_y2.tile([128, BT], MD, tag=f"y2_{m}")
                    drain(t, ps, b2s[:, m : m + 1], m + 1)
                    y2.append(t)

                # layer 3 (only the 8 useful output columns), sigmoid
                ps3 = pp_z3.tile([NQP, BT], F32, tag="z3")
                for kc in range(4):
                    nc.tensor.matmul(
                        ps3, w3s[:, kc, :], y2[kc], start=(kc == 0), stop=(kc == 3)
                    )
                qpT = p_qp.tile([NQP, BT], F32, tag="qpT")
                nc.scalar.activation(
                    out=qpT, in_=ps3, func=AF.Sigmoid, bias=b3s[:, 0:1], scale=1.0
                )

                # transpose to batch-major [128, 4 chunks, 8]; free the bank fast
                psq = pp_qpt.tile([128, 4, NQP], F32, tag="qpt")
                for c in range(4):
                    nc.tensor.transpose(
                        psq[:, c, :], qpT[:, c * 128 : (c + 1) * 128], id8[:]
                    )
                # closed-form 100-step update on [128, 4, 4] fp32
                q = psq[:, :, 0:4]
                p = psq[:, :, 4:8]
                TS = nc.vector.tensor_scalar

                u0b = p_cf.tile([128, 4, 4], F32, tag="u0b")
                nc.sync.dma_start(out=u0b, in_=u0_t[:, it])

                a = p_cf.tile([128, 4, 4], F32, tag="a")  # a = 1 - 2*lr*q
                nc.scalar.activation(out=a, in_=q, func=AF.Copy, bias=1.0, scale=-2.0 * LR)
                a2 = p_cf.tile([128, 4, 4], F32, tag="a2")
                nc.vector.tensor_mul(a2, a, a)
                a4 = p_cf.tile([128, 4, 4], F32, tag="a4")
                nc.vector.tensor_mul(a4, a2, a2)
                a8 = p_cf.tile([128, 4, 4], F32, tag="a8")
                nc.vector.tensor_mul(a8, a4, a4)
                a16 = p_cf.tile([128, 4, 4], F32, tag="a16")
                nc.vector.tensor_mul(a16, a8, a8)
                a32 = p_cf.tile([128, 4, 4], F32, tag="a32")
                nc.vector.tensor_mul(a32, a16, a16)
                a64 = p_cf.tile([128, 4, 4], F32, tag="a64")
                nc.vector.tensor_mul(a64, a32, a32)
                a96 = p_cf.tile([128, 4, 4], F32, tag="a96")
                nc.vector.tensor_mul(a96, a64, a32)
                A = p_cf.tile([128, 4, 4], F32, tag="A")
                nc.vector.tensor_mul(A, a96, a4)

                n1 = p_cf.tile([128, 4, 4], F32, tag="n1")  # 0.5*(1-A)
                nc.scalar.activation(out=n1, in_=A, func=AF.Copy, bias=0.5, scale=-0.5)
                rq = p_cf.tile([128, 4, 4], F32, tag="rq")
                nc.vector.reciprocal(rq, q)
                r = p_cf.tile([128, 4, 4], F32, tag="r")
                nc.vector.tensor_mul(r, p, rq)
                tt = p_cf.tile([128, 4, 4], F32, tag="tt")
                nc.vector.tensor_mul(tt, r, n1)
                mm = p_cf.tile([128, 4, 4], F32, tag="mm")
                nc.vector.tensor_mul(mm, A, u0b)
                uob = p_cf.tile([128, 4, 4], F32, tag="uob")
                nc.vector.tensor_sub(uob, mm, tt)
                nc.sync.dma_start(out=uo_t[:, it], in_=uob)
    nc.finalize()
    return nc


def _get_nc():
    if "nc" not in _CACHE:
        _CACHE["nc"] = _build_nc()
    return _CACHE["nc"]


def kernel(obs, x_init, u_init, W1, b1, W2, b2, W3, b3):
    obs = np.ascontiguousarray(np.asarray(obs, dtype=np.float32))
    u_init = np.ascontiguousarray(np.asarray(u_init, dtype=np.float32))
    W1 = np.asarray(W1, dtype=np.float32)
    W2 = np.asarray(W2, dtype=np.float32)
    W3 = np.asarray(W3, dtype=np.float32)
    b1 = np.asarray(b1, dtype=np.float32)
    b2 = np.asarray(b2, dtype=np.float32)
    b3 = np.asarray(b3, dtype=np.float32)

    # only columns 12:16 (q_u) and 28:32 (p_u) of the MLP head are used
    w3u = np.ascontiguousarray(np.concatenate([W3[:, 12:16], W3[:, 28:32]], axis=1))
    b3u = np.ascontiguousarray(np.concatenate([b3[12:16], b3[28:32]])[:, None])
    b1p = np.ascontiguousarray(b1.reshape(4, 128).T)  # [128, m] chunks
    b2p = np.ascontiguousarray(b2.reshape(4, 128).T)
    w1c = np.ascontiguousarray(W1)
    w2c = np.ascontiguousarray(W2)

    nc = _get_nc()
    in_maps = []
    for i in range(NCORES):
        in_maps.append(
            {
                "obs": obs[i * BPC : (i + 1) * BPC],
                "u0": u_init[i * BPC : (i + 1) * BPC],
                "w1": w1c,
                "w2": w2c,
                "w3": w3u,
                "b1": b1p,
                "b2": b2p,
                "b3": b3u,
            }
        )
    import os

    kw = {}
    if os.environ.get("BASSK_TRACE"):
        kw = {"trace": True, "tmpdir": os.environ.get("BASSK_TRACE_DIR") or None}
    res = run_bass_kernel_spmd(nc, in_maps, list(range(NCORES)), **kw)
    _CACHE["last_result"] = res
    out = np.concatenate([res.results[i]["uo"] for i in range(NCORES)], axis=0)
    return out.astype(np.float32)



# revision 9
# speedup vs baseline: 1.3610x; 1.3610x over previous
"""Trainium2 Bass kernel for nn_MPCActor: MLP (256->512->512->32, relu/relu/
sigmoid) + 100 SGD steps on u, solved in closed form (u <- a*u + b with
a = 1-2*lr*q_u has exact solution u_N = a^N u0 - 0.5*(p_u/q_u)*(1 - a^N)).

Data parallel over 8 NeuronCores: batch 32768 -> 4096 rows/core, weights
replicated. All matmul operands are fp8(e4m3): obs is pre-transposed and
pre-quantized on the host to [256, 4096] per core (feature-on-partition, so
no on-chip transposes), weights are scaled x64 on the host to dodge the fp8
subnormal cliff (undone in the drain scale) and packed in the DoubleRow
[K=128, 2, M] pair layout so each matmul contracts 256 rows per pass.

Layer 3 computes only the 8 useful W3 columns (q_u, p_u), zero-padded to 32
and col-tiled via tile_position so four batch tiles land concurrently in one
PSUM bank at partition offsets 0/32/64/96; one sigmoid drains all four, and
four [128,128] bf16 PE transposes flip a whole 4-tile round to batch-major.
The closed-form update then runs once per round on [128,4,4,4] APs.

PSUM drains alternate ACT/DVE; the closed-form's SBUF-only power chain runs
on the otherwise idle GpSimd. When the MLP biases are nonzero the kernel
falls back to per-chunk drains (ACT with exact bias; DVE/GpSimd chunks use
max(z,-64b)/64 whose constant deficit is folded into the next layer's bias
on the host).
"""

import numpy as np
import ml_dtypes

import concourse.bass as bass
import concourse.mybir as mybir
import concourse.tile as tile
from concourse import bacc, masks
from concourse.bass_utils import run_bass_kernel_spmd

NCORES = 8
BATCH = 32768
BPC = BATCH // NCORES  # 4096
OBS = 256
HID = 512
BT = 512               # batch tile (matmul moving free dim)
NT = BPC // BT         # 8 batch tiles per core
NR = NT // 4           # rounds of 4 tiles for layer 3
LR = 0.01
WS = 64.0              # weight pre-scale (host), undone in drain scale
F32 = mybir.dt.float32
BF16 = mybir.dt.bfloat16
FP8 = mybir.dt.float8e4
NPF8 = ml_dtypes.float8_e4m3

_CACHE = {}


def _build_nc(zero_bias: bool):
    nc = bacc.Bacc(
        trn_type="TRN2", target_bir_lowering=False, debug=False, num_devices=NCORES
    )
    obsd = nc.declare_dram_parameter("obsd", [OBS, BPC], FP8, isOutput=False).ap()
    w1d = nc.declare_dram_parameter("w1d", [128, 2, HID], FP8, isOutput=False).ap()
    w2d = nc.declare_dram_parameter("w2d", [128, 4, HID], FP8, isOutput=False).ap()
    w3d = nc.declare_dram_parameter("w3d", [128, 4, 32], FP8, isOutput=False).ap()
    bd = nc.declare_dram_parameter("bd", [128, 9], F32, isOutput=False).ap()
    u0d = nc.declare_dram_parameter("u0d", [NR, 128, 64], F32, isOutput=False).ap()
    uod = nc.declare_dram_parameter("uod", [NR, 128, 64], F32, isOutput=True).ap()

    AF = mybir.ActivationFunctionType
    ALU = mybir.AluOpType
    PM = mybir.MatmulPerfMode

    with tile.TileContext(nc) as tc:
        from contextlib import ExitStack

        with ExitStack() as ctx:
            singles = ctx.enter_context(tc.tile_pool(name="singles", bufs=1))
            p_obs = ctx.enter_context(tc.tile_pool(name="obs", bufs=3))
            p_y1 = ctx.enter_context(tc.tile_pool(name="y1", bufs=2))
            p_y2 = ctx.enter_context(tc.tile_pool(name="y2", bufs=NT))
            p_qs = ctx.enter_context(tc.tile_pool(name="qs", bufs=2))
            p_cf = ctx.enter_context(tc.tile_pool(name="cf", bufs=2))
            # PSUM budget 8 banks: pp12 4 + pp3 2 + ppt 2
            pp12 = ctx.enter_context(tc.tile_pool(name="pp12", bufs=2, space="PSUM"))
            pp3 = ctx.enter_context(tc.tile_pool(name="pp3", bufs=2, space="PSUM"))
            ppt = ctx.enter_context(tc.tile_pool(name="ppt", bufs=2, space="PSUM"))

            # ---- one-time loads (all pre-packed on host) ----
            w1s = singles.tile([128, 2, HID], FP8)
            nc.sync.dma_start(out=w1s, in_=w1d)
            obs_r = obsd.rearrange("(i p) n -> p i n", p=128)
            w2s = singles.tile([128, 4, HID], FP8)
            nc.sync.dma_start(out=w2s, in_=w2d)
            w3s = singles.tile([128, 4, 32], FP8)
            nc.sync.dma_start(out=w3s, in_=w3d)
            bs = singles.tile([128, 9], F32)
            nc.sync.dma_start(out=bs, in_=bd)
            ident = singles.tile([128, 128], BF16)
            masks.make_identity(nc, ident[:])

            def drain(eng, dst, src, bcol):
                # relu((z*64)/64 + b); ACT is exact, DVE computes
                # max(z*64, -64b)/64 = relu(z+b) - b (deficit pre-folded
                # into the next layer's bias on the host).
                if eng == "A":
                    bias = 0.0 if zero_bias else bs[:, bcol : bcol + 1]
                    nc.scalar.activation(
                        out=dst, in_=src, func=AF.Relu, bias=bias, scale=1.0 / WS
                    )
                else:
                    s1 = 0.0 if zero_bias else bs[:, bcol : bcol + 1]
                    nc.vector.tensor_scalar(dst, src, s1, 1.0 / WS, ALU.max, ALU.mult)

            y2r = []  # current round's y2 tiles
            for t in range(NT):
                ob = p_obs.tile([128, 2, BT], FP8, tag="obs")
                nc.sync.dma_start(out=ob, in_=obs_r[:, :, t * BT : (t + 1) * BT])

                # layer 1: z1' = W1'.T @ obs (DoubleRow: K=256 in one pass)
                y1 = p_y1.tile([128, 4, BT], FP8, tag="y1")
                if zero_bias:
                    for h in range(2):  # halves: m chunks (2h, 2h+1)
                        ps = pp12.tile([128, 2, BT], F32, tag="pp")
                        for m in (2 * h, 2 * h + 1):
                            nc.tensor.matmul(
                                ps[:, m - 2 * h, :],
                                w1s[:, :, m * 128 : (m + 1) * 128],
                                ob,
                                start=True,
                                stop=True,
                                perf_mode=PM.DoubleRow,
                            )
                        drain("AD"[h], y1[:, 2 * h : 2 * h + 2, :], ps, 0)
                else:
                    for m in range(4):
                        ps = pp12.tile([128, BT], F32, tag="pp1")
                        nc.tensor.matmul(
                            ps,
                            w1s[:, :, m * 128 : (m + 1) * 128],
                            ob,
                            start=True,
                            stop=True,
                            perf_mode=PM.DoubleRow,
                        )
                        drain("ADDA"[m], y1[:, m, :], ps, m)

                # layer 2: K=512 as 2 DoubleRow passes
                y2 = p_y2.tile([128, 4, BT], FP8, tag="y2")
                if zero_bias:
                    for h in range(2):
                        ps = pp12.tile([128, 2, BT], F32, tag="pp")
                        for m in (2 * h, 2 * h + 1):
                            for c in range(2):
                                nc.tensor.matmul(
                                    ps[:, m - 2 * h, :],
                                    w2s[:, 2 * c : 2 * c + 2, m * 128 : (m + 1) * 128],
                                    y1[:, 2 * c : 2 * c + 2, :],
                                    start=(c == 0),
                                    stop=(c == 1),
                                    perf_mode=PM.DoubleRow,
                                )
                        drain("AD"[h], y2[:, 2 * h : 2 * h + 2, :], ps, 0)
                else:
                    for m in range(4):
                        ps = pp12.tile([128, BT], F32, tag="pp1")
                        for c in range(2):
                            nc.tensor.matmul(
                                ps,
                                w2s[:, 2 * c : 2 * c + 2, m * 128 : (m + 1) * 128],
                                y1[:, 2 * c : 2 * c + 2, :],
                                start=(c == 0),
                                stop=(c == 1),
                                perf_mode=PM.DoubleRow,
                            )
                        drain("DAAD"[m], y2[:, m, :], ps, 4 + m)
                y2r.append(y2)

                if t % 4 != 3:
                    continue

                # ---- layer 3 for tiles 4r..4r+3, col-tiled into one bank ----
                r = t // 4
                ps3 = pp3.tile([128, BT], F32, tag="z3")
                for kc in range(4):
                    for g in range(4):
                        nc.tensor.matmul(
                            ps3[32 * g : 32 * (g + 1), :],
                            w3s[:, kc, :],
                            y2r[g][:, kc, :],
                            start=(kc == 0),
                            stop=(kc == 3),
                            tile_position=(0, 32 * g),
                        )
                y2r = []
                qs = p_qs.tile([128, BT], BF16, tag="qs")
                nc.scalar.activation(
                    out=qs, in_=ps3, func=AF.Sigmoid, bias=bs[:, 8:9], scale=1.0 / WS
                )
                # transpose whole round to batch-major: [128, (c, 32g+f)]
                pt = ppt.tile([128, 4, 4, 32], BF16, tag="pt")
                for c in range(4):
                    nc.tensor.transpose(
                        pt[:, c, :, :], qs[:, c * 128 : (c + 1) * 128], ident[:]
                    )
                q = pt[:, :, :, 0:4]
                p_ = pt[:, :, :, 4:8]

                u0b = p_cf.tile([128, 4, 4, 4], F32, tag="u0b")
                nc.sync.dma_start(out=u0b, in_=u0d[r])

                # closed form: u_N = A u0 - 0.5*(p/q)*(1-A), A = (1-2*lr*q)^100
                a = p_cf.tile([128, 4, 4, 4], F32, tag="a")
                nc.scalar.activation(
                    out=a, in_=q, func=AF.Copy, bias=1.0, scale=-2.0 * LR
                )
                pw = [None] * 7  # a^2,4,8,16,32,64 then a^96
                src = a
                for i in range(6):
                    pw[i] = p_cf.tile(
                        [128, 4, 4, 4], F32, name=f"pw{i}", tag=f"pw{i}"
                    )
                    nc.gpsimd.tensor_tensor(
                        out=pw[i], in0=src, in1=src, op=ALU.mult
                    )
                    src = pw[i]
                pw[6] = p_cf.tile([128, 4, 4, 4], F32, name="pw6", tag="pw6")
                nc.gpsimd.tensor_tensor(out=pw[6], in0=pw[5], in1=pw[4], op=ALU.mult)
                A = p_cf.tile([128, 4, 4, 4], F32, tag="A")
                nc.gpsimd.tensor_tensor(out=A, in0=pw[6], in1=pw[1], op=ALU.mult)

                n1 = p_cf.tile([128, 4, 4, 4], F32, tag="n1")  # 0.5*(1-A)
                nc.scalar.activation(
                    out=n1, in_=A, func=AF.Copy, bias=0.5, scale=-0.5
                )
                rq = p_cf.tile([128, 4, 4, 4], F32, tag="rq")
                nc.vector.reciprocal(rq, q)
                rr = p_cf.tile([128, 4, 4, 4], F32, tag="rr")
                nc.vector.tensor_tensor(out=rr, in0=p_, in1=rq, op=ALU.mult)
                tt = p_cf.tile([128, 4, 4, 4], F32, tag="tt")
                nc.vector.tensor_tensor(out=tt, in0=rr, in1=n1, op=ALU.mult)
                mm = p_cf.tile([128, 4, 4, 4], F32, tag="mm")
                nc.gpsimd.tensor_tensor(out=mm, in0=A, in1=u0b, op=ALU.mult)
                uob = p_cf.tile([128, 4, 4, 4], F32, tag="uob")
                nc.vector.tensor_tensor(out=uob, in0=mm, in1=tt, op=ALU.subtract)
                nc.sync.dma_start(out=uod[r], in_=uob)
    nc.finalize()
    return nc


def _get_nc(zero_bias: bool):
    key = ("nc", zero_bias)
    if key not in _CACHE:
        _CACHE[key] = _build_nc(zero_bias)
    return _CACHE[key]


def kernel(obs, x_init, u_init, W1, b1, W2, b2, W3, b3):
    obs = np.asarray(obs, dtype=np.float32)
    u_init = np.ascontiguousarray(np.asarray(u_init, dtype=np.float32))
    W1 = np.asarray(W1, dtype=np.float32)
    W2 = np.asarray(W2, dtype=np.float32)
    W3 = np.asarray(W3, dtype=np.float32)
    b1 = np.asarray(b1, dtype=np.float32)
    b2 = np.asarray(b2, dtype=np.float32)
    b3 = np.asarray(b3, dtype=np.float32)

    zero_bias = not (np.any(b1) or np.any(b2))

    # only columns 12:16 (q_u) and 28:32 (p_u) of the MLP head matter
    W3u = np.concatenate([W3[:, 12:16], W3[:, 28:32]], axis=1)  # [512, 8]
    b3u = np.concatenate([b3[12:16], b3[28:32]])  # [8]

    # fp8 packs; weights scaled x64 (drain scale undoes it)
    obs8 = obs.astype(NPF8)
    w1p = np.ascontiguousarray(
        (W1 * WS).astype(NPF8).reshape(2, 128, HID).transpose(1, 0, 2)
    )
    w2p = np.ascontiguousarray(
        (W2 * WS).astype(NPF8).reshape(4, 128, HID).transpose(1, 0, 2)
    )
    w3z = np.zeros((HID, 32), np.float32)
    w3z[:, :8] = W3u * WS
    w3p = np.ascontiguousarray(w3z.astype(NPF8).reshape(4, 128, 32).transpose(1, 0, 2))

    # bias pack + host-side deficit corrections for DVE-drained chunks
    if zero_bias:
        b2e = b2
        b3e = b3u
    else:
        # L1 DVE chunks m1,m2 store y1 - b1 on those features
        b2e = b2 + W2[128:384].T @ b1[128:384]
        # L2 DVE chunks m0,m3
        b3e = b3u + W3u[0:128].T @ b2e[0:128] + W3u[384:512].T @ b2e[384:512]
    bp = np.zeros((128, 9), np.float32)
    for m, e in enumerate("ADDA"):
        c = b1[m * 128 : (m + 1) * 128]
        bp[:, m] = c if e == "A" else -WS * c
    for m, e in enumerate("DAAD"):
        c = b2e[m * 128 : (m + 1) * 128]
        bp[:, 4 + m] = c if e == "A" else -WS * c
    for g in range(4):
        bp[32 * g : 32 * g + 8, 8] = b3e

    nc = _get_nc(zero_bias)
    in_maps = []
    for i in range(NCORES):
        sl = slice(i * BPC, (i + 1) * BPC)
        u0p = (
            u_init[sl]
            .reshape(NR, 4, 4, 128, 4)  # [r, g, c, n, j]
            .transpose(0, 3, 2, 1, 4)  # [r, n, c, g, j]
            .reshape(NR, 128, 64)
        )
        in_maps.append(
            {
                "obsd": np.ascontiguousarray(obs8[sl].T),
                "w1d": w1p,
                "w2d": w2p,
                "w3d": w3p,
                "bd": bp,
                "u0d": np.ascontiguousarray(u0p),
            }
        )
    import os

    kw = {}
    if os.environ.get("BASSK_TRACE"):
        kw = {"trace": True, "tmpdir": os.environ.get("BASSK_TRACE_DIR") or None}
    res = run_bass_kernel_spmd(nc, in_maps, list(range(NCORES)), **kw)
    _CACHE["last_result"] = res
    outs = []
    for i in range(NCORES):
        uop = res.results[i]["uod"].reshape(NR, 128, 4, 4, 4)
        outs.append(uop.transpose(0, 3, 2, 1, 4).reshape(BPC, 4))
    return np.concatenate(outs, axis=0).astype(np.float32)


# revision 12
# speedup vs baseline: 1.7804x; 1.3081x over previous
"""Trainium2 Bass kernel for nn_MPCActor: MLP (256->512->512->32, relu/relu/
sigmoid) + 100 SGD steps on u, solved in closed form (u <- a*u + b with
a = 1-2*lr*q_u has exact solution u_N = a^N u0 - 0.5*(p_u/q_u)*(1 - a^N)).

Data parallel over 8 NeuronCores: batch 32768 -> 4096 rows/core, weights
replicated. All matmul operands are fp8(e4m3): obs is pre-transposed and
pre-quantized on the host to [256, 4096] per core (feature-on-partition, so
no on-chip transposes), weights are scaled x64 on the host to dodge the fp8
subnormal cliff (undone in the drain scale) and packed in the DoubleRow
[K=128, 2, M] pair layout so each matmul contracts 256 rows per pass.

Layer 3 computes only the 8 useful W3 columns (q_u, p_u), zero-padded to 32
and col-tiled via tile_position so four batch tiles land concurrently in one
PSUM bank at partition offsets 0/32/64/96; one sigmoid drains all four, and
four [128,128] bf16 PE transposes flip a whole 4-tile round to batch-major.
The closed-form update then runs once per round on [128,4,4,4] APs.

PSUM drains alternate ACT/DVE; the closed-form's SBUF-only power chain runs
on the otherwise idle GpSimd. When the MLP biases are nonzero the kernel
falls back to per-chunk drains (ACT with exact bias; DVE/GpSimd chunks use
max(z,-64b)/64 whose constant deficit is folded into the next layer's bias
on the host).
"""

import numpy as np
import ml_dtypes

import concourse.bass as bass
import concourse.mybir as mybir
import concourse.tile as tile
from concourse import bacc, masks
from concourse.bass_utils import run_bass_kernel_spmd

NCORES = 8
BATCH = 32768
BPC = BATCH // NCORES  # 4096
OBS = 256
HID = 512
BT = 512               # batch tile (matmul moving free dim)
NT = BPC // BT         # 8 batch tiles per core
NR = NT // 4           # rounds of 4 tiles for layer 3
LR = 0.01
WS = 64.0              # weight pre-scale (host), undone in drain scale
F32 = mybir.dt.float32
BF16 = mybir.dt.bfloat16
FP8 = mybir.dt.float8e4
NPF8 = ml_dtypes.float8_e4m3

_CACHE = {}


def _build_nc(zero_bias: bool):
    nc = bacc.Bacc(
        trn_type="TRN2", target_bir_lowering=False, debug=False, num_devices=NCORES
    )
    obsd = nc.declare_dram_parameter("obsd", [OBS, BPC], FP8, isOutput=False).ap()
    w1d = nc.declare_dram_parameter("w1d", [128, 2, HID], FP8, isOutput=False).ap()
    w2d = nc.declare_dram_parameter("w2d", [128, 4, HID], FP8, isOutput=False).ap()
    w3d = nc.declare_dram_parameter("w3d", [128, 4, 32], FP8, isOutput=False).ap()
    bd = nc.declare_dram_parameter("bd", [128, 9], F32, isOutput=False).ap()
    u0d = nc.declare_dram_parameter("u0d", [NR, 128, 64], F32, isOutput=False).ap()
    uod = nc.declare_dram_parameter("uod", [NR, 128, 64], F32, isOutput=True).ap()

    AF = mybir.ActivationFunctionType
    ALU = mybir.AluOpType
    PM = mybir.MatmulPerfMode

    with tile.TileContext(nc) as tc:
        from contextlib import ExitStack

        with ExitStack() as ctx:
            singles = ctx.enter_context(tc.tile_pool(name="singles", bufs=1))
            p_obs = ctx.enter_context(tc.tile_pool(name="obs", bufs=3))
            p_y1 = ctx.enter_context(tc.tile_pool(name="y1", bufs=2))
            p_y2 = ctx.enter_context(tc.tile_pool(name="y2", bufs=NT))
            p_qs = ctx.enter_context(tc.tile_pool(name="qs", bufs=2))
            p_cf = ctx.enter_context(tc.tile_pool(name="cf", bufs=2))
            # PSUM budget 8 banks: pp12 6 + pp3 1 + ppt 1
            pp12 = ctx.enter_context(tc.tile_pool(name="pp12", bufs=3, space="PSUM"))
            pp3 = ctx.enter_context(tc.tile_pool(name="pp3", bufs=1, space="PSUM"))
            ppt = ctx.enter_context(tc.tile_pool(name="ppt", bufs=1, space="PSUM"))

            # ---- one-time loads (all pre-packed on host) ----
            # w1 + first obs tiles first: they gate the first matmul
            w1s = singles.tile([128, 2, HID], FP8)
            nc.sync.dma_start(out=w1s, in_=w1d)
            obs_r = obsd.rearrange("(i p) n -> p i n", p=128)
            obst = [None] * NT

            def obs_dma(k):
                obst[k] = p_obs.tile([128, 2, BT], FP8, name=f"ob{k}", tag="obs")
                nc.sync.dma_start(
                    out=obst[k], in_=obs_r[:, :, k * BT : (k + 1) * BT]
                )

            obs_dma(0)
            obs_dma(1)
            w2s = singles.tile([128, 4, HID], FP8)
            nc.sync.dma_start(out=w2s, in_=w2d)
            w3s = singles.tile([128, 4, 32], FP8)
            nc.sync.dma_start(out=w3s, in_=w3d)
            bs = singles.tile([128, 9], F32)
            nc.sync.dma_start(out=bs, in_=bd)
            ident = singles.tile([128, 128], BF16)
            masks.make_identity(nc, ident[:])

            def drain(eng, dst, src, bcol):
                # relu((z*64)/64 + b); ACT is exact, DVE computes
                # max(z*64, -64b)/64 = relu(z+b) - b (deficit pre-folded
                # into the next layer's bias on the host).
                if eng == "A":
                    bias = 0.0 if zero_bias else bs[:, bcol : bcol + 1]
                    nc.scalar.activation(
                        out=dst, in_=src, func=AF.Relu, bias=bias, scale=1.0 / WS
                    )
                else:
                    s1 = 0.0 if zero_bias else bs[:, bcol : bcol + 1]
                    nc.vector.tensor_scalar(dst, src, s1, 1.0 / WS, ALU.max, ALU.mult)

            y1_of = {}

            def L1(t):
                # layer 1: z1' = W1'.T @ obs (DoubleRow: K=256 in one pass)
                y1 = p_y1.tile([128, 4, BT], FP8, name=f"y1_{t}", tag="y1")
                ob = obst[t]
                if zero_bias:
                    for h in range(2):  # halves: m chunks (2h, 2h+1)
                        ps = pp12.tile([128, 2, BT], F32, name="ps1", tag="pp")
                        for m in (2 * h, 2 * h + 1):
                            nc.tensor.matmul(
                                ps[:, m - 2 * h, :],
                                w1s[:, :, m * 128 : (m + 1) * 128],
                                ob,
                                start=True,
                                stop=True,
                                perf_mode=PM.DoubleRow,
                            )
                        drain("AD"[h], y1[:, 2 * h : 2 * h + 2, :], ps, 0)
                else:
                    for m in range(4):
                        ps = pp12.tile([128, BT], F32, name="ps1", tag="pp1")
                        nc.tensor.matmul(
                            ps,
                            w1s[:, :, m * 128 : (m + 1) * 128],
                            ob,
                            start=True,
                            stop=True,
                            perf_mode=PM.DoubleRow,
                        )
                        drain("ADDA"[m], y1[:, m, :], ps, m)
                y1_of[t] = y1

            def L2(t):
                # layer 2: K=512 as 2 DoubleRow passes
                y1 = y1_of.pop(t)
                y2 = p_y2.tile([128, 4, BT], FP8, name=f"y2_{t}", tag="y2")
                if zero_bias:
                    for h in range(2):
                        ps = pp12.tile([128, 2, BT], F32, name="ps2", tag="pp")
                        for m in (2 * h, 2 * h + 1):
                            for c in range(2):
                                nc.tensor.matmul(
                                    ps[:, m - 2 * h, :],
                                    w2s[:, 2 * c : 2 * c + 2, m * 128 : (m + 1) * 128],
                                    y1[:, 2 * c : 2 * c + 2, :],
                                    start=(c == 0),
                                    stop=(c == 1),
                                    perf_mode=PM.DoubleRow,
                                )
                        drain("AD"[h], y2[:, 2 * h : 2 * h + 2, :], ps, 0)
                else:
                    for m in range(4):
                        ps = pp12.tile([128, BT], F32, name="ps2", tag="pp1")
                        for c in range(2):
                            nc.tensor.matmul(
                                ps,
                                w2s[:, 2 * c : 2 * c + 2, m * 128 : (m + 1) * 128],
                                y1[:, 2 * c : 2 * c + 2, :],
                                start=(c == 0),
                                stop=(c == 1),
                                perf_mode=PM.DoubleRow,
                            )
                        drain("DAAD"[m], y2[:, m, :], ps, 4 + m)
                y2r.append(y2)

            y2r = []  # current round's y2 tiles
            L1(0)
            for t in range(NT):
                if t + 2 < NT:
                    obs_dma(t + 2)
                if t + 1 < NT:
                    L1(t + 1)
                L2(t)

                if t % 4 != 3:
                    continue

                # ---- layer 3 for tiles 4r..4r+3, col-tiled into one bank ----
                r = t // 4
                ps3 = pp3.tile([128, BT], F32, tag="z3")
                for kc in range(4):
                    for g in range(4):
                        nc.tensor.matmul(
                            ps3[32 * g : 32 * (g + 1), :],
                            w3s[:, kc, :],
                            y2r[g][:, kc, :],
                            start=(kc == 0),
                            stop=(kc == 3),
                            tile_position=(0, 32 * g),
                        )
                y2r = []
                qs = p_qs.tile([128, BT], BF16, tag="qs")
                nc.scalar.activation(
                    out=qs, in_=ps3, func=AF.Sigmoid, bias=bs[:, 8:9], scale=1.0 / WS
                )
                # transpose whole round to batch-major: [128, (c, 32g+f)]
                pt = ppt.tile([128, 4, 4, 32], BF16, tag="pt")
                for c in range(4):
                    nc.tensor.transpose(
                        pt[:, c, :, :], qs[:, c * 128 : (c + 1) * 128], ident[:]
                    )
                q = pt[:, :, :, 0:4]
                p_ = pt[:, :, :, 4:8]

                u0b = p_cf.tile([128, 4, 4, 4], F32, tag="u0b")
                nc.sync.dma_start(out=u0b, in_=u0d[r])

                # closed form: u_N = A u0 - 0.5*(p/q)*(1-A), A = (1-2*lr*q)^100
                a = p_cf.tile([128, 4, 4, 4], F32, tag="a")
                nc.scalar.activation(
                    out=a, in_=q, func=AF.Copy, bias=1.0, scale=-2.0 * LR
                )
                pw = [None] * 7  # a^2,4,8,16,32,64 then a^96
                src = a
                for i in range(6):
                    pw[i] = p_cf.tile(
                        [128, 4, 4, 4], F32, name=f"pw{i}", tag=f"pw{i}"
                    )
                    nc.gpsimd.tensor_tensor(
                        out=pw[i], in0=src, in1=src, op=ALU.mult
                    )
                    src = pw[i]
                pw[6] = p_cf.tile([128, 4, 4, 4], F32, name="pw6", tag="pw6")
                nc.gpsimd.tensor_tensor(out=pw[6], in0=pw[5], in1=pw[4], op=ALU.mult)
                A = p_cf.tile([128, 4, 4, 4], F32, tag="A")
                nc.gpsimd.tensor_tensor(out=A, in0=pw[6], in1=pw[1], op=ALU.mult)

                n1 = p_cf.tile([128, 4, 4, 4], F32, tag="n1")  # 0.5*(1-A)
                nc.scalar.activation(
                    out=n1, in_=A, func=AF.Copy, bias=0.5, scale=-0.5
                )
                rq = p_cf.tile([128, 4, 4, 4], F32, tag="rq")
                nc.vector.reciprocal(rq, q)
                rr = p_cf.tile([128, 4, 4, 4], F32, tag="rr")
                nc.vector.tensor_tensor(out=rr, in0=p_, in1=rq, op=ALU.mult)
                tt = p_cf.tile([128, 4, 4, 4], F32, tag="tt")
                nc.vector.tensor_tensor(out=tt, in0=rr, in1=n1, op=ALU.mult)
                mm = p_cf.tile([128, 4, 4, 4], F32, tag="mm")
                nc.gpsimd.tensor_tensor(out=mm, in0=A, in1=u0b, op=ALU.mult)
                uob = p_cf.tile([128, 4, 4, 4], F32, tag="uob")
                nc.vector.tensor_tensor(out=uob, in0=mm, in1=tt, op=ALU.subtract)
                nc.sync.dma_start(out=uod[r], in_=uob)
    nc.finalize()
    return nc


def _get_nc(zero_bias: bool):
    key = ("nc", zero_bias)
    if key not in _CACHE:
        _CACHE[key] = _build_nc(zero_bias)
    return _CACHE[key]


def kernel(obs, x_init, u_init, W1, b1, W2, b2, W3, b3):
    obs = np.asarray(obs, dtype=np.float32)
    u_init = np.ascontiguousarray(np.asarray(u_init, dtype=np.float32))
    W1 = np.asarray(W1, dtype=np.float32)
    W2 = np.asarray(W2, dtype=np.float32)
    W3 = np.asarray(W3, dtype=np.float32)
    b1 = np.asarray(b1, dtype=np.float32)
    b2 = np.asarray(b2, dtype=np.float32)
    b3 = np.asarray(b3, dtype=np.float32)

    zero_bias = not (np.any(b1) or np.any(b2))

    # only columns 12:16 (q_u) and 28:32 (p_u) of the MLP head matter
    W3u = np.concatenate([W3[:, 12:16], W3[:, 28:32]], axis=1)  # [512, 8]
    b3u = np.concatenate([b3[12:16], b3[28:32]])  # [8]

    # fp8 packs; weights scaled x64 (drain scale undoes it)
    obs8 = obs.astype(NPF8)
    w1p = np.ascontiguousarray(
        (W1 * WS).astype(NPF8).reshape(2, 128, HID).transpose(1, 0, 2)
    )
    w2p = np.ascontiguousarray(
        (W2 * WS).astype(NPF8).reshape(4, 128, HID).transpose(1, 0, 2)
    )
    w3z = np.zeros((HID, 32), np.float32)
    w3z[:, :8] = W3u * WS
    w3p = np.ascontiguousarray(w3z.astype(NPF8).reshape(4, 128, 32).transpose(1, 0, 2))

    # bias pack + host-side deficit corrections for DVE-drained chunks
    if zero_bias:
        b2e = b2
        b3e = b3u
    else:
        # L1 DVE chunks m1,m2 store y1 - b1 on those features
        b2e = b2 + W2[128:384].T @ b1[128:384]
        # L2 DVE chunks m0,m3
        b3e = b3u + W3u[0:128].T @ b2e[0:128] + W3u[384:512].T @ b2e[384:512]
    bp = np.zeros((128, 9), np.float32)
    for m, e in enumerate("ADDA"):
        c = b1[m * 128 : (m + 1) * 128]
        bp[:, m] = c if e == "A" else -WS * c
    for m, e in enumerate("DAAD"):
        c = b2e[m * 128 : (m + 1) * 128]
        bp[:, 4 + m] = c if e == "A" else -WS * c
    for g in range(4):
        bp[32 * g : 32 * g + 8, 8] = b3e

    nc = _get_nc(zero_bias)
    in_maps = []
    for i in range(NCORES):
        sl = slice(i * BPC, (i + 1) * BPC)
        u0p = (
            u_init[sl]
            .reshape(NR, 4, 4, 128, 4)  # [r, g, c, n, j]
            .transpose(0, 3, 2, 1, 4)  # [r, n, c, g, j]
            .reshape(NR, 128, 64)
        )
        in_maps.append(
            {
                "obsd": np.ascontiguousarray(obs8[sl].T),
                "w1d": w1p,
                "w2d": w2p,
                "w3d": w3p,
                "bd": bp,
                "u0d": np.ascontiguousarray(u0p),
            }
        )
    import os

    kw = {}
    if os.environ.get("BASSK_TRACE"):
        kw = {"trace": True, "tmpdir": os.environ.get("BASSK_TRACE_DIR") or None}
    res = run_bass_kernel_spmd(nc, in_maps, list(range(NCORES)), **kw)
    _CACHE["last_result"] = res
    outs = []
    for i in range(NCORES):
        uop = res.results[i]["uod"].reshape(NR, 128, 4, 4, 4)
        outs.append(uop.transpose(0, 3, 2, 1, 4).reshape(BPC, 4))
    return np.concatenate(outs, axis=0).astype(np.float32)
